# revision 5
# baseline (speedup 1.0000x reference)
"""Bezier2Image Trainium2 kernel (Bass/Tile, 8-core data parallel).

Computation per sample b:
  ctrl = x[b].reshape(160, 4, 2); pts = T @ ctrl  -> 4800 (curve, t) points
  gX[p, w] = exp(-(bX_w - X_p)^2 / ALPHA), gY likewise  (separable splat)
  out[b] = min(gX^T @ gY, 1)   (contraction over the 4800 points)

Device mapping (per core, 16 samples), v2 — ACT-walled design:
  - points in 40 chunks of 120 (4 curves x 30 samples), partition dim =
    point-within-chunk
  - nkxy[p, (c, t)] = -KS * pts: ONE fp32 matmul (Wc @ staged) into PSUM
  - ds[p, c, t, w] = KS*bX_w - KS*pts  (bf16): one broadcast tensor_tensor,
    split DVE (56 of 80 (c,t)-units) / GPSIMD (24 units) to keep both
    under the ACT wall; GPSIMD reads a DVE-copied SBUF mirror of nkxy
  - gaussians: ONE ACT pass per sample over [120, 4800]:
    Derivative_Erf(x) = (2/sqrt(pi)) * exp(-x^2); the (4/pi) factor on
    gX*gY is undone in the epilogue.  ACT busy = 16*(224+4800)/1.2 ~ 67us
    = the roofline for this kernel (only ACT can exp; 9.2M exps/core).
  - accumulation: 40 bf16 matmuls [120x60]^T @ [120x60] into one PSUM bank
  - epilogue on DVE: min(res * pi/4, 1), DMA out
"""

import numpy as np

N = 30
W = 60
LENGTH = 160
ALPHA = 2e-4
B = 128
NCORES = 8
BPC = B // NCORES  # samples per core
KS = float(1.0 / np.sqrt(ALPHA))
NCH = 40  # chunks per sample
PCH = 120  # points per chunk (4 curves x 30)
# ds work split over the 80 (chunk, coord) units: DVE computes [0, DVE_U) in
# two instructions (split at HALF_U so the first ACT half can start early),
# GPSIMD computes [DVE_U, 80).
HALF_U = 40
DVE_U = 54

_state = {}


def _bezier_T():
    t = np.arange(N, dtype=np.float64) / N
    t = 2.0 * t**3 - 3.0 * t**2 + 2.0 * t
    t3 = t**3
    T = np.stack(
        [t3, 3.0 * (t**2 - t3), 3.0 * (t3 - 2.0 * t**2 + t), (1.0 - t) ** 3],
        axis=1,
    )
    return T  # [N, 4] float64


def build_nc(loop_n=1, sim_safe=False):
    from contextlib import ExitStack

    import concourse.bacc as bacc
    import concourse.mybir as mybir
    import concourse.tile as tile

    fp32 = mybir.dt.float32
    bf16 = mybir.dt.bfloat16
    AF = mybir.ActivationFunctionType

    # Bacc (not plain Bass): its compile() pass splits multi-sem waits into
    # event-semaphore instructions — walrus codegen allows only one sync wait
    # per compute instruction.
    nc = bacc.Bacc()
    x_in = nc.declare_dram_parameter("x", [BPC, LENGTH, 8], fp32, isOutput=False)
    out_d = nc.declare_dram_parameter("out", [BPC, W, W], fp32, isOutput=True)

    # Constants.
    T = _bezier_T()  # [30, 4]
    q = np.arange(PCH)
    # Wc[(dl, k), q] = -KS * T[q % 30, k] if q // 30 == dl else 0.
    # One matmul Wc.T @ staged then computes -KS * pts for a whole
    # sample: nkXY[q, (c,t)] = sum_{dl,k} Wc[(dl,k), q] * x[b, 4c+dl, 2k+t].
    Wc_np = np.zeros((16, PCH), np.float32)
    for dl in range(4):
        for k in range(4):
            row = np.where(q // N == dl, -KS * T[q % N, k], 0.0)
            Wc_np[dl * 4 + k] = row.astype(np.float32)
    bxk_np = np.broadcast_to(
        (KS * np.arange(W, dtype=np.float64) / W).astype(np.float32), (128, W)
    ).copy()

    Wc_d = nc.inline_tensor(Wc_np, "Wc")
    bxk_d = nc.inline_tensor(bxk_np, "bxk")

    with ExitStack() as ctx:
        tc = ctx.enter_context(tile.TileContext(nc))
        consts = ctx.enter_context(tc.tile_pool(name="consts", bufs=1))
        small = ctx.enter_context(tc.tile_pool(name="small", bufs=4))
        big = ctx.enter_context(tc.tile_pool(name="big", bufs=4))
        psum = ctx.enter_context(tc.tile_pool(name="psum", bufs=3, space="PSUM"))
        psum_pts = ctx.enter_context(tc.tile_pool(name="psum_pts", bufs=3, space="PSUM"))
        outp = ctx.enter_context(tc.tile_pool(name="outp", bufs=6))

        Wc = consts.tile([16, PCH], fp32)
        nc.sync.dma_start(out=Wc, in_=Wc_d[:, :])
        bxk = consts.tile([128, W], fp32)
        nc.sync.dma_start(out=bxk, in_=bxk_d[:, :])

        loop_ctx = tc.For_i(0, loop_n, 1) if loop_n > 1 else None
        if loop_ctx is not None:
            ctx.enter_context(loop_ctx)

        for b in range(BPC):
            # staged[(dl,k), c, t] = x[b, 4c+dl, 2k+t]
            staged = small.tile([16, NCH, 2], fp32)
            xb = x_in[b].rearrange("(c dl) (k t) -> dl k c t", dl=4, t=2)
            for t in range(2):
                nc.sync.dma_start(
                    out=staged[:, :, t],
                    in_=xb[:, :, :, t].rearrange("dl k c -> (dl k) c"),
                )
            # nkxy[q, (c, t)] = -KS * pts[l(q,c), n(q), t]
            nkxy = psum_pts.tile([PCH, NCH * 2], fp32, name=f"nkxy_{b}", tag="nkxy")
            nc.tensor.matmul(nkxy, Wc, staged.rearrange("k c t -> k (c t)"))
            # SBUF mirror for GPSIMD (it cannot read PSUM).
            nkxy_sb = small.tile([PCH, NCH * 2], fp32, name=f"nkxysb_{b}", tag="nkxy_sb")
            nc.vector.tensor_copy(nkxy_sb, nkxy)

            # ds[q, u, w] = KS*bX_w - KS*pts  for the 80 (c,t) units u
            ds = big.tile([PCH, NCH * 2, W], bf16, name=f"ds_{b}", tag="ds")
            for lo, hi in ((0, HALF_U), (HALF_U, DVE_U)):
                nc.vector.tensor_add(
                    ds[:, lo:hi],
                    bxk[:PCH].unsqueeze(1).broadcast_to([PCH, hi - lo, W]),
                    nkxy[:, lo:hi].unsqueeze(2).broadcast_to([PCH, hi - lo, W]),
                )
            nc.gpsimd.tensor_add(
                ds[:, DVE_U:],
                bxk[:PCH].unsqueeze(1).broadcast_to([PCH, NCH * 2 - DVE_U, W]),
                nkxy_sb[:, DVE_U:].unsqueeze(2).broadcast_to([PCH, NCH * 2 - DVE_U, W]),
            )

            # Two ACT half-passes: g = (2/sqrt(pi)) * exp(-ds^2).  Two (not
            # one) so the first half's res matmuls start ~2.2us earlier and
            # PE idle gaps stay under the ~3.4us HAM re-throttle window.
            g = big.tile([PCH, NCH * 2, W], bf16, name=f"g_{b}", tag="g")
            if sim_safe:
                # CoreSim lacks Derivative_Erf: equivalent two-op path.
                d2 = big.tile([PCH, NCH * 2, W], bf16, name=f"d2_{b}", tag="d2")
                nc.vector.tensor_mul(d2, ds, ds)
                nc.scalar.activation(g, d2, AF.Exp, scale=-1.0)
                nc.vector.tensor_scalar_mul(g, g, float(2.0 / np.sqrt(np.pi)))
            else:
                nc.scalar.activation(g[:, :HALF_U], ds[:, :HALF_U], AF.Derivative_Erf)
                nc.scalar.activation(g[:, HALF_U:], ds[:, HALF_U:], AF.Derivative_Erf)

            res = psum.tile([W, W], fp32)
            gv = g.rearrange("q (c t) w -> q c t w", t=2)
            for c in range(NCH):
                nc.tensor.matmul(
                    res,
                    gv[:, c, 0],
                    gv[:, c, 1],
                    start=(c == 0),
                    stop=(c == NCH - 1),
                )

            res_sb = outp.tile([W, W], fp32, name=f"rs_{b}", tag="res_sb")
            # res carries the (2/sqrt(pi))^2 factor from Derivative_Erf:
            # undo with *pi/4, then clamp.
            nc.vector.tensor_scalar(
                res_sb,
                res,
                float(np.pi / 4.0),
                1.0,
                op0=mybir.AluOpType.mult,
                op1=mybir.AluOpType.min,
            )
            nc.sync.dma_start(out=out_d[b], in_=res_sb)

    nc.compile()
    return nc


def kernel(x):
    import os

    x = np.ascontiguousarray(x, dtype=np.float32)
    assert x.shape == (B, LENGTH, 8), x.shape
    if "nc" not in _state:
        _state["nc"] = build_nc()
    from concourse.bass_utils import run_bass_kernel_spmd

    in_maps = [{"x": x[i * BPC : (i + 1) * BPC]} for i in range(NCORES)]
    trace = bool(os.environ.get("BEZIER_TRACE"))
    res = run_bass_kernel_spmd(
        _state["nc"], in_maps, core_ids=list(range(NCORES)), trace=trace
    )
    _state["last_results"] = res
    return np.concatenate([r["out"] for r in res.results], axis=0)


# revision 11
# speedup vs baseline: 1.2261x; 1.2261x over previous
"""Bezier2Image Trainium2 kernel (Bass/Tile, 8-core data parallel).

Computation per sample b:
  ctrl = x[b].reshape(160, 4, 2); pts = T @ ctrl  -> 4800 (curve, t) points
  gX[p, w] = exp(-(bX_w - X_p)^2 / ALPHA), gY likewise  (separable splat)
  out[b] = min(gX^T @ gY, 1)   (contraction over the 4800 points)

Device mapping (per core, 16 samples), v2 — ACT-walled design:
  - points in 40 chunks of 120 (4 curves x 30 samples), partition dim =
    point-within-chunk
  - nkxy[p, (c, t)] = -KS * pts: ONE fp32 matmul (Wc @ staged) into PSUM
  - ds[p, c, t, w] = KS*bX_w - KS*pts  (bf16): one broadcast tensor_tensor,
    split DVE (56 of 80 (c,t)-units) / GPSIMD (24 units) to keep both
    under the ACT wall; GPSIMD reads a DVE-copied SBUF mirror of nkxy
  - gaussians: ONE ACT pass per sample over [120, 4800]:
    Derivative_Erf(x) = (2/sqrt(pi)) * exp(-x^2); the (4/pi) factor on
    gX*gY is undone in the epilogue.  ACT busy = 16*(224+4800)/1.2 ~ 67us
    = the roofline for this kernel (only ACT can exp; 9.2M exps/core).
  - accumulation: 40 bf16 matmuls [120x60]^T @ [120x60] into one PSUM bank
  - epilogue on DVE: min(res * pi/4, 1), DMA out
"""

import numpy as np

N = 30
W = 60
LENGTH = 160
ALPHA = 2e-4
B = 128
NCORES = 8
BPC = B // NCORES  # samples per core
KS = float(1.0 / np.sqrt(ALPHA))
NCH = 40  # chunks per sample
PCH = 120  # points per chunk (4 curves x 30)
# ds work split over the 80 (chunk, coord) units: DVE computes [0, DVE_U),
# GPSIMD computes [DVE_U, 80).  56/24 balances both engines just under the
# ACT wall (measured best; all-DVE and 60/20 are worse — GPSIMD genuinely
# overlaps despite sharing one SBUF port with DVE).
HALF_U = 40
DVE_U = 56

_state = {}


def _bezier_T():
    t = np.arange(N, dtype=np.float64) / N
    t = 2.0 * t**3 - 3.0 * t**2 + 2.0 * t
    t3 = t**3
    T = np.stack(
        [t3, 3.0 * (t**2 - t3), 3.0 * (t3 - 2.0 * t**2 + t), (1.0 - t) ** 3],
        axis=1,
    )
    return T  # [N, 4] float64


def build_nc(loop_n=1, sim_safe=False, ablate=(), dve_u=DVE_U, split_act=False):
    ablate = frozenset(ablate)
    from contextlib import ExitStack

    import concourse.bacc as bacc
    import concourse.mybir as mybir
    import concourse.tile as tile

    fp32 = mybir.dt.float32
    bf16 = mybir.dt.bfloat16
    AF = mybir.ActivationFunctionType

    # Bacc (not plain Bass): its compile() pass splits multi-sem waits into
    # event-semaphore instructions — walrus codegen allows only one sync wait
    # per compute instruction.
    nc = bacc.Bacc()
    x_in = nc.declare_dram_parameter("x", [BPC, LENGTH, 8], fp32, isOutput=False)
    out_d = nc.declare_dram_parameter("out", [BPC, W, W], fp32, isOutput=True)

    # Constants.
    T = _bezier_T()  # [30, 4]
    q = np.arange(PCH)
    # Wc[(dl, k), q] = -KS * T[q % 30, k] if q // 30 == dl else 0.
    # One matmul Wc.T @ staged then computes -KS * pts for a whole
    # sample: nkXY[q, (c,t)] = sum_{dl,k} Wc[(dl,k), q] * x[b, 4c+dl, 2k+t].
    Wc_np = np.zeros((16, PCH), np.float32)
    for dl in range(4):
        for k in range(4):
            row = np.where(q // N == dl, -KS * T[q % N, k], 0.0)
            Wc_np[dl * 4 + k] = row.astype(np.float32)
    bxk_np = np.broadcast_to(
        (KS * np.arange(W, dtype=np.float64) / W).astype(np.float32), (128, W)
    ).copy()

    Wc_d = nc.inline_tensor(Wc_np, "Wc")
    bxk_d = nc.inline_tensor(bxk_np, "bxk")

    with ExitStack() as ctx:
        tc = ctx.enter_context(tile.TileContext(nc))
        consts = ctx.enter_context(tc.tile_pool(name="consts", bufs=1))
        small = ctx.enter_context(tc.tile_pool(name="small", bufs=4))
        big = ctx.enter_context(tc.tile_pool(name="big", bufs=4))
        psum = ctx.enter_context(tc.tile_pool(name="psum", bufs=3, space="PSUM"))
        psum_pts = ctx.enter_context(tc.tile_pool(name="psum_pts", bufs=3, space="PSUM"))
        outp = ctx.enter_context(tc.tile_pool(name="outp", bufs=6))

        Wc = consts.tile([16, PCH], fp32)
        nc.sync.dma_start(out=Wc, in_=Wc_d[:, :])
        bxk = consts.tile([128, W], fp32)
        nc.sync.dma_start(out=bxk, in_=bxk_d[:, :])

        loop_ctx = tc.For_i(0, loop_n, 1) if loop_n > 1 else None
        if loop_ctx is not None:
            ctx.enter_context(loop_ctx)

        for b in range(BPC):
            # staged[(dl,k), c, t] = x[b, 4c+dl, 2k+t]
            staged = small.tile([16, NCH, 2], fp32)
            xb = x_in[b].rearrange("(c dl) (k t) -> dl k c t", dl=4, t=2)
            for t in range(2):
                nc.sync.dma_start(
                    out=staged[:, :, t],
                    in_=xb[:, :, :, t].rearrange("dl k c -> (dl k) c"),
                )
            # nkxy[q, (c, t)] = -KS * pts[l(q,c), n(q), t]
            nkxy = psum_pts.tile([PCH, NCH * 2], fp32, name=f"nkxy_{b}", tag="nkxy")
            nc.tensor.matmul(nkxy, Wc, staged.rearrange("k c t -> k (c t)"))
            if dve_u < NCH * 2:
                # SBUF mirror for GPSIMD (it cannot read PSUM).
                nkxy_sb = small.tile(
                    [PCH, NCH * 2], fp32, name=f"nkxysb_{b}", tag="nkxy_sb"
                )
                nc.vector.tensor_copy(nkxy_sb, nkxy)

            # ds[q, u, w] = KS*bX_w - KS*pts  for the 80 (c,t) units u
            ds = big.tile([PCH, NCH * 2, W], bf16, name=f"ds_{b}", tag="ds")
            if "ds" not in ablate:
                dve_ranges = (
                    ((0, HALF_U), (HALF_U, dve_u)) if split_act else ((0, dve_u),)
                )
                for lo, hi in dve_ranges:
                    nc.vector.tensor_add(
                        ds[:, lo:hi],
                        bxk[:PCH].unsqueeze(1).broadcast_to([PCH, hi - lo, W]),
                        nkxy[:, lo:hi].unsqueeze(2).broadcast_to([PCH, hi - lo, W]),
                    )
                if dve_u < NCH * 2:
                    nc.gpsimd.tensor_add(
                        ds[:, dve_u:],
                        bxk[:PCH].unsqueeze(1).broadcast_to([PCH, NCH * 2 - dve_u, W]),
                        nkxy_sb[:, dve_u:].unsqueeze(2).broadcast_to(
                            [PCH, NCH * 2 - dve_u, W]
                        ),
                    )

            # ACT pass(es): g = (2/sqrt(pi)) * exp(-ds^2).
            g = big.tile([PCH, NCH * 2, W], bf16, name=f"g_{b}", tag="g")
            if sim_safe:
                # CoreSim lacks Derivative_Erf: equivalent two-op path.
                d2 = big.tile([PCH, NCH * 2, W], bf16, name=f"d2_{b}", tag="d2")
                nc.vector.tensor_mul(d2, ds, ds)
                nc.scalar.activation(g, d2, AF.Exp, scale=-1.0)
                nc.vector.tensor_scalar_mul(g, g, float(2.0 / np.sqrt(np.pi)))
            elif "act" in ablate:
                g = ds
            elif split_act:
                nc.scalar.activation(g[:, :HALF_U], ds[:, :HALF_U], AF.Derivative_Erf)
                nc.scalar.activation(g[:, HALF_U:], ds[:, HALF_U:], AF.Derivative_Erf)
            else:
                nc.scalar.activation(g, ds, AF.Derivative_Erf)

            res = psum.tile([W, W], fp32)
            gv = g.rearrange("q (c t) w -> q c t w", t=2)
            mm_chunks = (0, NCH - 1) if "mm" in ablate else tuple(range(NCH))
            for i, c in enumerate(mm_chunks):
                nc.tensor.matmul(
                    res,
                    gv[:, c, 0],
                    gv[:, c, 1],
                    start=(i == 0),
                    stop=(i == len(mm_chunks) - 1),
                )

            res_sb = outp.tile([W, W], fp32, name=f"rs_{b}", tag="res_sb")
            # res carries the (2/sqrt(pi))^2 factor from Derivative_Erf:
            # undo with *pi/4, then clamp.
            nc.vector.tensor_scalar(
                res_sb,
                res,
                float(np.pi / 4.0),
                1.0,
                op0=mybir.AluOpType.mult,
                op1=mybir.AluOpType.min,
            )
            nc.sync.dma_start(out=out_d[b], in_=res_sb)

    nc.compile()
    return nc


def kernel(x):
    import os

    x = np.ascontiguousarray(x, dtype=np.float32)
    assert x.shape == (B, LENGTH, 8), x.shape
    if "nc" not in _state:
        _state["nc"] = build_nc()
    from concourse.bass_utils import run_bass_kernel_spmd

    in_maps = [{"x": x[i * BPC : (i + 1) * BPC]} for i in range(NCORES)]
    trace = bool(os.environ.get("BEZIER_TRACE"))
    res = run_bass_kernel_spmd(
        _state["nc"], in_maps, core_ids=list(range(NCORES)), trace=trace
    )
    _state["last_results"] = res
    return np.concatenate([r["out"] for r in res.results], axis=0)


# revision 20
# speedup vs baseline: 1.3484x; 1.0997x over previous
"""Bezier2Image Trainium2 kernel (Bass/Tile, 8-core data parallel).

Computation per sample b:
  ctrl = x[b].reshape(160, 4, 2); pts = T @ ctrl  -> 4800 (curve, t) points
  gX[p, w] = exp(-(bX_w - X_p)^2 / ALPHA), gY likewise  (separable splat)
  out[b] = min(gX^T @ gY, 1)   (contraction over the 4800 points)

Device mapping (per core, 16 samples), v2 — ACT-walled design:
  - points in 40 chunks of 120 (4 curves x 30 samples), partition dim =
    point-within-chunk
  - nkxy[p, (c, t)] = -KS * pts: ONE fp32 matmul (Wc @ staged) into PSUM
  - ds[p, c, t, w] = KS*bX_w - KS*pts  (bf16): one broadcast tensor_tensor,
    split DVE (56 of 80 (c,t)-units) / GPSIMD (24 units) to keep both
    under the ACT wall; GPSIMD reads a DVE-copied SBUF mirror of nkxy
  - gaussians: ONE ACT pass per sample over [120, 4800]:
    Derivative_Erf(x) = (2/sqrt(pi)) * exp(-x^2); the (4/pi) factor on
    gX*gY is undone in the epilogue.  ACT busy = 16*(224+4800)/1.2 ~ 67us
    = the roofline for this kernel (only ACT can exp; 9.2M exps/core).
  - accumulation: 40 bf16 matmuls [120x60]^T @ [120x60] into one PSUM bank
  - epilogue on DVE: min(res * pi/4, 1), DMA out
"""

import numpy as np

N = 30
W = 60
LENGTH = 160
ALPHA = 2e-4
B = 128
NCORES = 8
BPC = B // NCORES  # samples per core
KS = float(1.0 / np.sqrt(ALPHA))
NCH = 40  # chunks per sample
PCH = 120  # points per chunk (4 curves x 30)
# ds work split over the 80 (chunk, coord) units: DVE computes [0, DVE_U),
# GPSIMD computes [DVE_U, 80).  56/24 balances both engines just under the
# ACT wall (measured best; all-DVE and 60/20 are worse — GPSIMD genuinely
# overlaps despite sharing one SBUF port with DVE).
HALF_U = 40
DVE_U = 56

_state = {}


def _bezier_T():
    t = np.arange(N, dtype=np.float64) / N
    t = 2.0 * t**3 - 3.0 * t**2 + 2.0 * t
    t3 = t**3
    T = np.stack(
        [t3, 3.0 * (t**2 - t3), 3.0 * (t3 - 2.0 * t**2 + t), (1.0 - t) ** 3],
        axis=1,
    )
    return T  # [N, 4] float64


def build_nc(
    loop_n=1, sim_safe=False, ablate=(), dve_u=DVE_U, split_act=False, pair_act=False
):
    ablate = frozenset(ablate)
    from contextlib import ExitStack

    import concourse.bacc as bacc
    import concourse.mybir as mybir
    import concourse.tile as tile

    fp32 = mybir.dt.float32
    bf16 = mybir.dt.bfloat16
    AF = mybir.ActivationFunctionType

    # Bacc (not plain Bass): its compile() pass splits multi-sem waits into
    # event-semaphore instructions — walrus codegen allows only one sync wait
    # per compute instruction.
    nc = bacc.Bacc()
    x_in = nc.declare_dram_parameter("x", [BPC, LENGTH, 8], fp32, isOutput=False)
    out_d = nc.declare_dram_parameter("out", [BPC, W, W], fp32, isOutput=True)

    # Constants.
    T = _bezier_T()  # [30, 4]
    q = np.arange(PCH)
    # Wc[(dl, k), q] = -KS * T[q % 30, k] if q // 30 == dl else 0.
    # One matmul Wc.T @ staged then computes -KS * pts for a whole
    # sample: nkXY[q, (c,t)] = sum_{dl,k} Wc[(dl,k), q] * x[b, 4c+dl, 2k+t].
    Wc_np = np.zeros((16, PCH), np.float32)
    for dl in range(4):
        for k in range(4):
            row = np.where(q // N == dl, -KS * T[q % N, k], 0.0)
            Wc_np[dl * 4 + k] = row.astype(np.float32)
    bxk_np = np.broadcast_to(
        (KS * np.arange(W, dtype=np.float64) / W).astype(np.float32), (128, W)
    ).copy()

    Wc_d = nc.inline_tensor(Wc_np, "Wc")
    bxk_d = nc.inline_tensor(bxk_np, "bxk")

    with ExitStack() as ctx:
        tc = ctx.enter_context(tile.TileContext(nc))
        consts = ctx.enter_context(tc.tile_pool(name="consts", bufs=1))
        small = ctx.enter_context(tc.tile_pool(name="small", bufs=4))
        big = ctx.enter_context(tc.tile_pool(name="big", bufs=4))
        psum = ctx.enter_context(tc.tile_pool(name="psum", bufs=3, space="PSUM"))
        psum_pts = ctx.enter_context(tc.tile_pool(name="psum_pts", bufs=3, space="PSUM"))
        outp = ctx.enter_context(tc.tile_pool(name="outp", bufs=6))

        Wc = consts.tile([16, PCH], fp32)
        nc.sync.dma_start(out=Wc, in_=Wc_d[:, :])
        bxk = consts.tile([128, W], fp32)
        nc.sync.dma_start(out=bxk, in_=bxk_d[:, :])

        loop_ctx = tc.For_i(0, loop_n, 1) if loop_n > 1 else None
        if loop_ctx is not None:
            ctx.enter_context(loop_ctx)

        _deferred_tail = None
        for b in range(BPC):
            # staged[(dl,k), c, t] = x[b, 4c+dl, 2k+t]
            staged = small.tile([16, NCH, 2], fp32)
            nc.sync.dma_start(
                out=staged,
                in_=x_in[b].rearrange("(c dl) (k t) -> (dl k) c t", dl=4, t=2),
            )
            # nkxy[q, (c, t)] = -KS * pts[l(q,c), n(q), t]
            nkxy = psum_pts.tile([PCH, NCH * 2], fp32, name=f"nkxy_{b}", tag="nkxy")
            nc.tensor.matmul(nkxy, Wc, staged.rearrange("k c t -> k (c t)"))
            if dve_u < NCH * 2:
                # SBUF mirror for GPSIMD (it cannot read PSUM).
                nkxy_sb = small.tile(
                    [PCH, NCH * 2], fp32, name=f"nkxysb_{b}", tag="nkxy_sb"
                )
                nc.vector.tensor_copy(nkxy_sb, nkxy)

            # ds/g tiles are allocated per PAIR of samples; ACT instructions
            # slice them flexibly: one big [120, 9600] instruction per middle
            # pair (amortizes the ~190ns per-instruction SBUF bubble), but
            # the first sample is split into halves (pipeline fills ~2us
            # sooner) and the last sample too (its res matmuls overlap the
            # second ACT half, shortening the drain).
            if pair_act:
                if b % 2 == 0:
                    ds_pair = big.tile(
                        [PCH, 2, NCH * 2, W], bf16, name=f"ds_{b}", tag="ds"
                    )
                    g_pair = big.tile(
                        [PCH, 2, NCH * 2, W], bf16, name=f"g_{b}", tag="g"
                    )
                    _state_pair = (ds_pair, g_pair)
                else:
                    ds_pair, g_pair = _state_pair
                ds = ds_pair[:, b % 2]
                g = g_pair[:, b % 2]
            else:
                ds = big.tile([PCH, NCH * 2, W], bf16, name=f"ds_{b}", tag="ds")
                g = big.tile([PCH, NCH * 2, W], bf16, name=f"g_{b}", tag="g")

            if "ds" not in ablate:
                split_ds = split_act or b == 0 or b == BPC - 1
                dve_ranges = (
                    ((0, HALF_U), (HALF_U, dve_u)) if split_ds else ((0, dve_u),)
                )
                for lo, hi in dve_ranges:
                    nc.vector.tensor_add(
                        ds[:, lo:hi],
                        bxk[:PCH].unsqueeze(1).broadcast_to([PCH, hi - lo, W]),
                        nkxy[:, lo:hi].unsqueeze(2).broadcast_to([PCH, hi - lo, W]),
                    )
                if dve_u < NCH * 2:
                    nc.gpsimd.tensor_add(
                        ds[:, dve_u:],
                        bxk[:PCH].unsqueeze(1).broadcast_to([PCH, NCH * 2 - dve_u, W]),
                        nkxy_sb[:, dve_u:].unsqueeze(2).broadcast_to(
                            [PCH, NCH * 2 - dve_u, W]
                        ),
                    )

            # ACT pass(es): g = (2/sqrt(pi)) * exp(-ds^2).  For merged middle
            # pairs the single pair-wide instruction is emitted on the ODD
            # sample, and the even sample's tail is deferred past it (Tile
            # dependency tracking is emission-ordered).
            act_emitted = True
            if sim_safe:
                # CoreSim lacks Derivative_Erf: equivalent two-op path.
                d2 = big.tile([PCH, NCH * 2, W], bf16, name=f"d2_{b}", tag="d2")
                nc.vector.tensor_mul(d2, ds, ds)
                nc.scalar.activation(g, d2, AF.Exp, scale=-1.0)
                nc.vector.tensor_scalar_mul(g, g, float(2.0 / np.sqrt(np.pi)))
            elif "act" in ablate:
                g = ds
            elif b == 0 or b == BPC - 1 or split_act:
                nc.scalar.activation(g[:, :HALF_U], ds[:, :HALF_U], AF.Derivative_Erf)
                nc.scalar.activation(g[:, HALF_U:], ds[:, HALF_U:], AF.Derivative_Erf)
            elif not pair_act or b == 1 or b == BPC - 2:
                # full-sample instruction
                nc.scalar.activation(g, ds, AF.Derivative_Erf)
            elif b % 2 == 1:
                # merged: one instruction covers this sample and the previous
                nc.scalar.activation(g_pair, ds_pair, AF.Derivative_Erf)
            else:
                act_emitted = False  # covered by the pair ACT next iteration

            def emit_tail(bb, g_s):
                res = psum.tile([W, W], fp32, name=f"res_{bb}", tag="res")
                gv = g_s.rearrange("q (c t) w -> q c t w", t=2)
                mm_chunks = (0, NCH - 1) if "mm" in ablate else tuple(range(NCH))
                for i, c in enumerate(mm_chunks):
                    nc.tensor.matmul(
                        res,
                        gv[:, c, 0],
                        gv[:, c, 1],
                        start=(i == 0),
                        stop=(i == len(mm_chunks) - 1),
                    )
                res_sb = outp.tile([W, W], fp32, name=f"rs_{bb}", tag="res_sb")
                # res carries the (2/sqrt(pi))^2 factor from Derivative_Erf:
                # undo with *pi/4, then clamp.
                nc.vector.tensor_scalar(
                    res_sb,
                    res,
                    float(np.pi / 4.0),
                    1.0,
                    op0=mybir.AluOpType.mult,
                    op1=mybir.AluOpType.min,
                )
                nc.sync.dma_start(out=out_d[bb], in_=res_sb)

            if act_emitted:
                if _deferred_tail is not None:
                    emit_tail(*_deferred_tail)
                    _deferred_tail = None
                emit_tail(b, g)
            else:
                _deferred_tail = (b, g)

    nc.compile()
    return nc


def kernel(x):
    import os

    x = np.ascontiguousarray(x, dtype=np.float32)
    assert x.shape == (B, LENGTH, 8), x.shape
    if "nc" not in _state:
        _state["nc"] = build_nc()
    from concourse.bass_utils import run_bass_kernel_spmd

    in_maps = [{"x": x[i * BPC : (i + 1) * BPC]} for i in range(NCORES)]
    trace = bool(os.environ.get("BEZIER_TRACE"))
    res = run_bass_kernel_spmd(
        _state["nc"], in_maps, core_ids=list(range(NCORES)), trace=trace
    )
    _state["last_results"] = res
    return np.concatenate([r["out"] for r in res.results], axis=0)
